# revision 1
# baseline (speedup 1.0000x reference)
"""Trainium2 Bass kernel: MultiHeadSelfAttention (LayerNorm -> QKV -> masked
softmax attention -> output projection).

Problem shapes: B=4, S=2048, D=512, H=8, DK=64, fp32 I/O.

Sharding: 8 cores = 4 batches x 2 query-halves. Each core computes the full
K/V for its batch and attention outputs for its 1024 queries; no cross-core
communication. SPMD trick: the token order of each core's input is permuted on
the host so that the core's queries are always tokens 0..1023 (one static
program for all cores; attention is permutation-equivariant over keys as long
as the key-padding mask is permuted consistently).

On-device dataflow (per core):
  - LayerNorm in token-major [128 tok, 512] tiles (bn_stats/bn_aggr +
    fused (x-mu)*rstd tensor_scalar, bf16 out), PE-transpose to xnT [d, tok].
  - QKV projections in bf16 (gamma/beta folded into weights/biases on host,
    which is exact here since gamma=1, beta=0).
    QT/KT are produced d-major ([head_dim, tok]) so scores contract over
    head_dim directly; V is produced token-major and stored interleaved with a
    ones column ([V_h | 1] per head) so P@[V|1] yields both P@V and the
    softmax denominator l = sum_k exp.
  - scoresT[k, q] per head via PE; one ACT op fuses scale (1/8), key-padding
    mask (additive -1e30 bias per partition) and exp (softmax without
    max-subtraction is safe here: |scores| <~ 8).
  - PV with the exp'd scores chunk as the matmul stationary; all 8 query
    tiles accumulate in 3 PSUM banks; normalize by 1/l on evacuation.
  - PE-transpose attention output, output projection, + bias, DMA out.
"""

import math

import numpy as np

import concourse.bass as bass
import concourse.tile as tile
from concourse import bacc, mybir
from concourse.bass_utils import run_bass_kernel_spmd
from concourse.masks import make_identity

B, S, D, H, DK = 4, 2048, 512, 8, 64
P = 128                 # partitions
NQ = 1024               # queries per core
NT = S // P             # 16 token tiles / key chunks
DC = D // P             # 4 d-chunks
NQT = NQ // P           # 8 query tiles
PAIRS = H // 2          # 4 head pairs
F32 = mybir.dt.float32
BF16 = mybir.dt.bfloat16
NEG = -1.0e30
DEBUG = False


def _emit(tc: tile.TileContext, ctx):
    nc = tc.nc

    x_d = nc.dram_tensor("x", [S, D], F32, kind="ExternalInput")
    wq_d = nc.dram_tensor("wq", [D, D], F32, kind="ExternalInput")
    wk_d = nc.dram_tensor("wk", [D, D], F32, kind="ExternalInput")
    wv_d = nc.dram_tensor("wv", [D, D], F32, kind="ExternalInput")
    wo_d = nc.dram_tensor("wo", [D, D], F32, kind="ExternalInput")
    bq_d = nc.dram_tensor("bq", [P, DC], F32, kind="ExternalInput")
    bk_d = nc.dram_tensor("bk", [P, DC], F32, kind="ExternalInput")
    bo_d = nc.dram_tensor("bo", [D], F32, kind="ExternalInput")
    mb_d = nc.dram_tensor("maskb", [P, NT], F32, kind="ExternalInput")
    y_d = nc.dram_tensor("y", [NQ, D], F32, kind="ExternalOutput")
    if DEBUG:
        dbg = {
            "dbg_xnT": nc.dram_tensor("dbg_xnT", [P, DC * S], BF16, kind="ExternalOutput"),
            "dbg_qT": nc.dram_tensor("dbg_qT", [P, DC * NQ], BF16, kind="ExternalOutput"),
            "dbg_kT": nc.dram_tensor("dbg_kT", [P, DC * S], BF16, kind="ExternalOutput"),
            "dbg_vaug": nc.dram_tensor("dbg_vaug", [P, NT * 520], BF16, kind="ExternalOutput"),
            "dbg_pt": nc.dram_tensor("dbg_pt", [2, P, 2 * NQ], BF16, kind="ExternalOutput"),
            "dbg_pv": nc.dram_tensor("dbg_pv", [3, P, 512], F32, kind="ExternalOutput"),
            "dbg_attno": nc.dram_tensor("dbg_attno", [P, NQT * D], BF16, kind="ExternalOutput"),
        }

    consts = ctx.enter_context(tc.tile_pool(name="consts", bufs=1))
    big = ctx.enter_context(tc.tile_pool(name="big", bufs=1))
    stage = ctx.enter_context(tc.tile_pool(name="stage", bufs=3))
    xnp = ctx.enter_context(tc.tile_pool(name="xnp", bufs=3))
    stats = ctx.enter_context(tc.tile_pool(name="stats", bufs=6))
    ptp = ctx.enter_context(tc.tile_pool(name="ptp", bufs=4))
    rlp = ctx.enter_context(tc.tile_pool(name="rlp", bufs=6))
    yout = ctx.enter_context(tc.tile_pool(name="yout", bufs=3))

    ident = consts.tile([P, P], BF16, tag="ident")
    make_identity(nc, ident)
    bq_sb = consts.tile([P, DC], F32, tag="bq")
    nc.sync.dma_start(bq_sb, bq_d[:, :])
    bk_sb = consts.tile([P, DC], F32, tag="bk")
    nc.sync.dma_start(bk_sb, bk_d[:, :])
    mb_sb = consts.tile([P, NT], F32, tag="mb")
    nc.sync.dma_start(mb_sb, mb_d[:, :])
    eps_sb = consts.tile([P, 1], F32, tag="eps")
    nc.vector.memset(eps_sb, 1e-5)
    bo_sb = consts.tile([P, D], F32, tag="bo")
    bo_ap = bo_d[:]
    nc.sync.dma_start(
        bo_sb, bass.AP(tensor=bo_ap.tensor, offset=bo_ap.offset, ap=[[0, P], [1, D]])
    )

    # persistent bf16 operands
    w_sb = {}
    for name, d in (("wq", wq_d), ("wk", wk_d), ("wv", wv_d), ("wo", wo_d)):
        w_sb[name] = big.tile([P, DC, D], BF16, tag=f"{name}_sb", name=f"{name}_sb")
    xnT = big.tile([P, DC, S], BF16, tag="xnT")
    qT = big.tile([P, DC, NQ], BF16, tag="qT")
    kT = big.tile([P, DC, S], BF16, tag="kT")
    vaug = big.tile([P, NT, 8 * 65], BF16, tag="vaug")
    attno = big.tile([P, NQT, D], BF16, tag="attno")
    outT = big.tile([P, DC, NQ], BF16, tag="outT")

    # ---------------- phase A/B: LN, transposes, projections ----------------
    with tc.tile_pool(name="projA", bufs=3, space="PSUM") as projA:
        # weights load + cast
        for name, d in (("wq", wq_d), ("wk", wk_d), ("wv", wv_d), ("wo", wo_d)):
            for c in range(DC):
                ws = stage.tile([P, D], F32, tag="wstage")
                nc.sync.dma_start(ws, d[c * P : (c + 1) * P, :])
                nc.vector.tensor_copy(out=w_sb[name][:, c, :], in_=ws)

        # LayerNorm + transpose to xnT
        for t in range(NT):
            xt = stage.tile([P, D], F32, tag="xstage")
            nc.sync.dma_start(xt, x_d[t * P : (t + 1) * P, :])
            st = stats.tile([P, 6], F32, tag="st")
            nc.vector.bn_stats(out=st, in_=xt)
            mv = stats.tile([P, 2], F32, tag="mv")
            nc.vector.bn_aggr(out=mv, in_=st)
            sd = stats.tile([P, 1], F32, tag="sd")
            nc.scalar.activation(
                out=sd, in_=mv[:, 1:2], func=mybir.ActivationFunctionType.Sqrt,
                bias=eps_sb,
            )
            rr = stats.tile([P, 1], F32, tag="rr")
            nc.vector.reciprocal(out=rr, in_=sd)
            xn = xnp.tile([P, D], BF16, tag="xn")
            nc.vector.tensor_scalar(
                out=xn, in0=xt, scalar1=mv[:, 0:1], scalar2=rr,
                op0=mybir.AluOpType.subtract, op1=mybir.AluOpType.mult,
            )
            pt4 = projA.tile([P, D], BF16, tag="ppsum")
            for c in range(DC):
                nc.tensor.transpose(
                    pt4[:, c * P : (c + 1) * P], xn[:, c * P : (c + 1) * P], ident
                )
            nc.vector.tensor_copy(
                out=xnT[:, :, t * P : (t + 1) * P],
                in_=pt4[:].rearrange("p (c q) -> p c q", c=DC),
            )

        # QT projection (queries = tokens 0..NQ-1)
        for dqc in range(DC):
            for qg in range(NQ // 512):
                ps = projA.tile([P, 512], F32, tag="ppsum")
                for dc in range(DC):
                    nc.tensor.matmul(
                        ps,
                        w_sb["wq"][:, dc, dqc * P : (dqc + 1) * P],
                        xnT[:, dc, qg * 512 : (qg + 1) * 512],
                        start=(dc == 0), stop=(dc == DC - 1),
                    )
                nc.vector.tensor_scalar_add(
                    out=qT[:, dqc, qg * 512 : (qg + 1) * 512], in0=ps,
                    scalar1=bq_sb[:, dqc : dqc + 1],
                )
        # KT projection (all tokens)
        for dkc in range(DC):
            for kg in range(S // 512):
                ps = projA.tile([P, 512], F32, tag="ppsum")
                for dc in range(DC):
                    nc.tensor.matmul(
                        ps,
                        w_sb["wk"][:, dc, dkc * P : (dkc + 1) * P],
                        xnT[:, dc, kg * 512 : (kg + 1) * 512],
                        start=(dc == 0), stop=(dc == DC - 1),
                    )
                nc.vector.tensor_scalar_add(
                    out=kT[:, dkc, kg * 512 : (kg + 1) * 512], in0=ps,
                    scalar1=bk_sb[:, dkc : dkc + 1],
                )
        # V projection, token-major, interleaved [V_h | 1] per head
        for t in range(NT):
            ps = projA.tile([P, 512], F32, tag="ppsum")
            for dc in range(DC):
                nc.tensor.matmul(
                    ps,
                    xnT[:, dc, t * P : (t + 1) * P],
                    w_sb["wv"][:, dc, :],
                    start=(dc == 0), stop=(dc == DC - 1),
                )
            vslot = vaug[:, t, :].rearrange("p (h c) -> p h c", h=H)
            nc.vector.tensor_copy(
                out=vslot[:, :, 0:DK],
                in_=ps[:].rearrange("p (h c) -> p h c", h=H),
            )
            nc.vector.memset(vslot[:, :, DK : DK + 1], 1.0)
        if DEBUG:
            nc.sync.dma_start(dbg["dbg_xnT"][:, :], xnT[:, :, :].rearrange("p c s -> p (c s)"))
            nc.sync.dma_start(dbg["dbg_qT"][:, :], qT[:, :, :].rearrange("p c s -> p (c s)"))
            nc.sync.dma_start(dbg["dbg_kT"][:, :], kT[:, :, :].rearrange("p c s -> p (c s)"))
            nc.sync.dma_start(dbg["dbg_vaug"][:, :], vaug[:, :, :].rearrange("p c s -> p (c s)"))

    # ---------------- phase C/D: attention ----------------
    with (
        tc.tile_pool(name="scp", bufs=2, space="PSUM") as scp,
        tc.tile_pool(name="pvp", bufs=3, space="PSUM") as pvp,
    ):
        for p in range(PAIRS):
            pvb = [
                pvp.tile([P, 512], F32, tag="pvb", name=f"pvb{p}_{j}")
                for j in range(3)
            ]
            pts = []
            for c in range(NT):
                pt = ptp.tile([P, 2 * NQ], BF16, tag="pt")
                pts.append(pt)
                for hs in range(2):
                    sc = scp.tile([P, NQ], F32, tag="sc")
                    for qg in range(NQ // 512):
                        nc.tensor.matmul(
                            sc[:, qg * 512 : (qg + 1) * 512],
                            kT[hs * DK : (hs + 1) * DK, p, c * P : (c + 1) * P],
                            qT[hs * DK : (hs + 1) * DK, p, qg * 512 : (qg + 1) * 512],
                            start=True, stop=True,
                        )
                    nc.scalar.activation(
                        out=pt[:, hs * NQ : (hs + 1) * NQ], in_=sc,
                        func=mybir.ActivationFunctionType.Exp,
                        bias=mb_sb[:, c : c + 1], scale=1.0 / math.sqrt(DK),
                    )
                if DEBUG and p == 0 and c < 2:
                    nc.sync.dma_start(dbg["dbg_pt"][c, :, :], pt[:, :])
                # PV matmuls for the previous chunk (keeps PE busy while ACT
                # works on this chunk's exp)
                if c > 0:
                    _pv_chunk(nc, pts[c - 1], vaug, pvb, p, c - 1)
            _pv_chunk(nc, pts[NT - 1], vaug, pvb, p, NT - 1)
            if DEBUG and p == 0:
                for j in range(3):
                    dbgpv = yout.tile([P, 512], F32, tag="yt", name=f"dbgpv{j}")
                    nc.vector.tensor_copy(out=dbgpv, in_=pvb[j][:, :])
                    nc.sync.dma_start(dbg["dbg_pv"][j, :, :], dbgpv)

            # evacuate + normalize
            for qt in range(NQT):
                bank = pvb[qt // 3]
                off = (qt % 3) * 130
                rl = rlp.tile([P, 2], F32, tag="rl")
                for hs in range(2):
                    nc.vector.reciprocal(
                        out=rl[:, hs : hs + 1],
                        in_=bank[:, off + hs * 65 + DK : off + hs * 65 + DK + 1],
                    )
                for hs in range(2):
                    nc.vector.tensor_scalar_mul(
                        out=attno[:, qt, (2 * p + hs) * DK : (2 * p + hs + 1) * DK],
                        in0=bank[:, off + hs * 65 : off + hs * 65 + DK],
                        scalar1=rl[:, hs : hs + 1],
                    )

    if DEBUG:
        nc.sync.dma_start(
            dbg["dbg_attno"][:, :], attno[:, :, :].rearrange("p c s -> p (c s)")
        )

    # ---------------- phase E: transpose + output projection ----------------
    with tc.tile_pool(name="projE", bufs=3, space="PSUM") as projE:
        for qt in range(NQT):
            pe = projE.tile([P, D], BF16, tag="epsum")
            for c in range(DC):
                nc.tensor.transpose(
                    pe[:, c * P : (c + 1) * P], attno[:, qt, c * P : (c + 1) * P],
                    ident,
                )
            nc.vector.tensor_copy(
                out=outT[:, :, qt * P : (qt + 1) * P],
                in_=pe[:].rearrange("p (c q) -> p c q", c=DC),
            )
        for qt in range(NQT):
            po = projE.tile([P, D], F32, tag="epsum")
            for dc in range(DC):
                nc.tensor.matmul(
                    po,
                    outT[:, dc, qt * P : (qt + 1) * P],
                    w_sb["wo"][:, dc, :],
                    start=(dc == 0), stop=(dc == DC - 1),
                )
            yt = yout.tile([P, D], F32, tag="yt")
            nc.vector.tensor_tensor(
                out=yt, in0=po, in1=bo_sb, op=mybir.AluOpType.add
            )
            nc.sync.dma_start(y_d[qt * P : (qt + 1) * P, :], yt)


def _pv_chunk(nc, pt, vaug, pvb, p, c):
    """P@[V|1] matmuls for chunk c of head-pair p: 8 query tiles x 2 heads,
    accumulated over chunks into the packed PSUM banks."""
    for qt in range(NQT):
        bank = pvb[qt // 3]
        off = (qt % 3) * 130
        for hs in range(2):
            h = 2 * p + hs
            # start=True clears has_written for the WHOLE bank, so only the
            # first packed region per bank may use it; the others rely on
            # overwrite-when-bit-clear for their first chunk.
            nc.tensor.matmul(
                bank[:, off + hs * 65 : off + (hs + 1) * 65],
                pt[:, hs * NQ + qt * P : hs * NQ + (qt + 1) * P],
                vaug[:, c, h * 65 : (h + 1) * 65],
                start=(c == 0 and qt % 3 == 0 and hs == 0),
                stop=(c == NT - 1),
                skip_group_check=True,
            )


_NC = None


def _get_nc():
    global _NC
    if _NC is None:
        from contextlib import ExitStack

        nc = bacc.Bacc(None, target_bir_lowering=False)
        with tile.TileContext(nc) as tc, ExitStack() as ctx:
            _emit(tc, ctx)
        nc.compile()
        _NC = nc
    return _NC


def kernel(
    inputs, input_lengths, pos_embed, ln_gamma, ln_beta,
    Wq, bq, Wk, bk, Wv, bv, Wo, bo,
):
    x = np.ascontiguousarray(np.asarray(inputs, np.float32))
    lengths = np.asarray(input_lengths, np.int32)
    g = np.asarray(ln_gamma, np.float32)
    be = np.asarray(ln_beta, np.float32)
    Wq = np.asarray(Wq, np.float32); bq = np.asarray(bq, np.float32)
    Wk = np.asarray(Wk, np.float32); bk = np.asarray(bk, np.float32)
    Wv = np.asarray(Wv, np.float32); bv = np.asarray(bv, np.float32)
    Wo = np.asarray(Wo, np.float32); bo = np.asarray(bo, np.float32)

    # Fold LayerNorm affine into the projections (exact: LN(x) = xh*g + be
    # with xh = (x-mu)*rstd, so LN(x)@W.T + b = xh@(g[:,None]*W.T) + (be@W.T + b)).
    wq_h = np.ascontiguousarray(g[:, None] * Wq.T)
    wk_h = np.ascontiguousarray(g[:, None] * Wk.T)
    wv_h = np.ascontiguousarray(g[:, None] * Wv.T)
    wo_h = np.ascontiguousarray(Wo.T)
    bq_h = np.ascontiguousarray((be @ Wq.T + bq).reshape(DC, P).T)
    bk_h = np.ascontiguousarray((be @ Wk.T + bk).reshape(DC, P).T)
    # V bias (incl. beta term) passes through softmax (rows sum to 1) and is
    # folded into the output-projection bias.
    bv_h = be @ Wv.T + bv
    bo_h = np.ascontiguousarray(bo + bv_h @ Wo.T)

    maskb = np.where(np.arange(S)[None, :] < lengths[:, None], 0.0, NEG).astype(
        np.float32
    )

    nc = _get_nc()
    in_maps = []
    core_assign = []
    for b in range(B):
        for gq in range(2):
            order = np.r_[gq * NQ : (gq + 1) * NQ, (1 - gq) * NQ : (2 - gq) * NQ]
            in_maps.append(
                {
                    "x": np.ascontiguousarray(x[b][order]),
                    "wq": wq_h, "wk": wk_h, "wv": wv_h, "wo": wo_h,
                    "bq": bq_h, "bk": bk_h, "bo": bo_h,
                    "maskb": np.ascontiguousarray(maskb[b][order].reshape(NT, P).T),
                }
            )
            core_assign.append((b, gq))

    global _LAST_IN_MAPS
    _LAST_IN_MAPS = in_maps
    res = run_bass_kernel_spmd(nc, in_maps, core_ids=list(range(8)))

    y = np.empty((B, S, D), np.float32)
    for i, (b, gq) in enumerate(core_assign):
        y[b, gq * NQ : (gq + 1) * NQ] = res.results[i]["y"]
    return y



# revision 3
# speedup vs baseline: 1.2131x; 1.2131x over previous
"""Trainium2 Bass kernel: MultiHeadSelfAttention (LayerNorm -> QKV -> masked
softmax attention -> output projection).

Problem shapes: B=4, S=2048, D=512, H=8, DK=64, fp32 I/O.

Sharding: 8 cores = 4 batches x 2 query-halves. Each core computes the full
K/V for its batch and attention outputs for its 1024 queries; no cross-core
communication. SPMD trick: the token order of each core's input is permuted on
the host so that the core's queries are always tokens 0..1023 (one static
program for all cores; attention is permutation-equivariant over keys as long
as the key-padding mask is permuted consistently).

Schedule (v2) is built around two facts measured on the v1 trace:
  - ScalarE exp over all scores is ~147us of unavoidable ACT time; it must
    run gapless, so the two per-chunk score PSUM tiles (one per head-half)
    are persistent and exp(c) never waits on anything but its own 4 matmuls.
  - The PE HAM clock-gate held the whole attention phase at 1.2 GHz in v1
    because exp<->scores serialization left PE idle gaps; v2 keeps the PE
    queue dense (scores(c+1) + PV(c-1) + next pair's QK projections all
    interleave under the exp stream).

PSUM budget (8 banks): sc0 (2) + sc1 (2) + PV accumulators (3) + proj (1).
"""

import math

import numpy as np
from ml_dtypes import bfloat16 as np_bf16

import concourse.bass as bass
import concourse.tile as tile
from concourse import bacc, mybir
from concourse.bass_utils import run_bass_kernel_spmd
from concourse.masks import make_identity

B, S, D, H, DK = 4, 2048, 512, 8, 64
P = 128                 # partitions
NQ = 1024               # queries per core
NT = S // P             # 16 token tiles / key chunks
DC = D // P             # 4 d-chunks
NQT = NQ // P           # 8 query tiles
PAIRS = H // 2          # 4 head pairs
F32 = mybir.dt.float32
BF16 = mybir.dt.bfloat16
NEG = -1.0e30


def _emit(tc: tile.TileContext, ctx):
    nc = tc.nc

    x_d = nc.dram_tensor("x", [S, D], F32, kind="ExternalInput")
    wq_d = nc.dram_tensor("wq", [D, D], BF16, kind="ExternalInput")
    wk_d = nc.dram_tensor("wk", [D, D], BF16, kind="ExternalInput")
    wv_d = nc.dram_tensor("wv", [D, D], BF16, kind="ExternalInput")
    wo_d = nc.dram_tensor("wo", [D, D], BF16, kind="ExternalInput")
    bq_d = nc.dram_tensor("bq", [P, DC], F32, kind="ExternalInput")
    bk_d = nc.dram_tensor("bk", [P, DC], F32, kind="ExternalInput")
    bo_d = nc.dram_tensor("bo", [D], F32, kind="ExternalInput")
    mb_d = nc.dram_tensor("maskb", [P, NT], F32, kind="ExternalInput")
    y_d = nc.dram_tensor("y", [NQ, D], F32, kind="ExternalOutput")

    consts = ctx.enter_context(tc.tile_pool(name="consts", bufs=1))
    big = ctx.enter_context(tc.tile_pool(name="big", bufs=1))
    stage = ctx.enter_context(tc.tile_pool(name="stage", bufs=3))
    xnp = ctx.enter_context(tc.tile_pool(name="xnp", bufs=3))
    stats = ctx.enter_context(tc.tile_pool(name="stats", bufs=6))
    ptp = ctx.enter_context(tc.tile_pool(name="ptp", bufs=4))
    rlp = ctx.enter_context(tc.tile_pool(name="rlp", bufs=6))
    yout = ctx.enter_context(tc.tile_pool(name="yout", bufs=3))

    ident = consts.tile([P, P], BF16, tag="ident")
    make_identity(nc, ident)
    bq_sb = consts.tile([P, DC], F32, tag="bq")
    nc.sync.dma_start(bq_sb, bq_d[:, :])
    bk_sb = consts.tile([P, DC], F32, tag="bk")
    nc.sync.dma_start(bk_sb, bk_d[:, :])
    mb_sb = consts.tile([P, NT], F32, tag="mb")
    nc.sync.dma_start(mb_sb, mb_d[:, :])
    eps_sb = consts.tile([P, 1], F32, tag="eps")
    nc.vector.memset(eps_sb, 1e-5)
    bo_sb = consts.tile([P, D], F32, tag="bo")
    bo_ap = bo_d[:]
    nc.sync.dma_start(
        bo_sb, bass.AP(tensor=bo_ap.tensor, offset=bo_ap.offset, ap=[[0, P], [1, D]])
    )

    # persistent bf16 operands (weights arrive bf16 from the host)
    w_sb = {}
    for name, d in (("wq", wq_d), ("wk", wk_d), ("wv", wv_d), ("wo", wo_d)):
        w = big.tile([P, DC, D], BF16, tag=f"{name}_sb", name=f"{name}_sb")
        for c in range(DC):
            nc.sync.dma_start(w[:, c, :], d[c * P : (c + 1) * P, :])
        w_sb[name] = w
    xnT = big.tile([P, DC, S], BF16, tag="xnT")
    qT = big.tile([P, DC, NQ], BF16, tag="qT")
    kT = big.tile([P, DC, S], BF16, tag="kT")
    vaug = big.tile([P, NT, 8 * 65], BF16, tag="vaug")
    attno = big.tile([P, NQT, D], BF16, tag="attno")
    outT = big.tile([P, DC, NQ], BF16, tag="outT")

    def qk_proj_groups(p, pool):
        """Emit-closures for pair p's Q and K projections (d-chunk p)."""
        groups = []

        def q_group(qg):
            def emit():
                ps = pool.tile([P, 512], F32, tag="pp", name=f"qps{p}_{qg}")
                for dc in range(DC):
                    nc.tensor.matmul(
                        ps,
                        w_sb["wq"][:, dc, p * P : (p + 1) * P],
                        xnT[:, dc, qg * 512 : (qg + 1) * 512],
                        start=(dc == 0), stop=(dc == DC - 1),
                    )
                nc.vector.tensor_scalar_add(
                    out=qT[:, p, qg * 512 : (qg + 1) * 512], in0=ps,
                    scalar1=bq_sb[:, p : p + 1],
                )
            return emit

        def k_group(kg):
            def emit():
                ps = pool.tile([P, 512], F32, tag="pp", name=f"kps{p}_{kg}")
                for dc in range(DC):
                    nc.tensor.matmul(
                        ps,
                        w_sb["wk"][:, dc, p * P : (p + 1) * P],
                        xnT[:, dc, kg * 512 : (kg + 1) * 512],
                        start=(dc == 0), stop=(dc == DC - 1),
                    )
                nc.vector.tensor_scalar_add(
                    out=kT[:, p, kg * 512 : (kg + 1) * 512], in0=ps,
                    scalar1=bk_sb[:, p : p + 1],
                )
            return emit

        for qg in range(NQ // 512):
            groups.append(q_group(qg))
        for kg in range(S // 512):
            groups.append(k_group(kg))
        return groups

    # ---------------- phase A: LayerNorm + transpose + V projection ----------
    with tc.tile_pool(name="lnp", bufs=2, space="PSUM") as lnp:
        for t in range(NT):
            xt = stage.tile([P, D], F32, tag="xstage")
            nc.sync.dma_start(xt, x_d[t * P : (t + 1) * P, :])
            st = stats.tile([P, 6], F32, tag="st")
            nc.vector.bn_stats(out=st, in_=xt)
            mv = stats.tile([P, 2], F32, tag="mv")
            nc.vector.bn_aggr(out=mv, in_=st)
            sd = stats.tile([P, 1], F32, tag="sd")
            nc.scalar.activation(
                out=sd, in_=mv[:, 1:2], func=mybir.ActivationFunctionType.Sqrt,
                bias=eps_sb,
            )
            rr = stats.tile([P, 1], F32, tag="rr")
            nc.vector.reciprocal(out=rr, in_=sd)
            xn = xnp.tile([P, D], BF16, tag="xn")
            nc.vector.tensor_scalar(
                out=xn, in0=xt, scalar1=mv[:, 0:1], scalar2=rr,
                op0=mybir.AluOpType.subtract, op1=mybir.AluOpType.mult,
            )
            pt4 = lnp.tile([P, D], BF16, tag="tps")
            for c in range(DC):
                nc.tensor.transpose(
                    pt4[:, c * P : (c + 1) * P], xn[:, c * P : (c + 1) * P], ident
                )
            nc.scalar.copy(
                out=xnT[:, :, t * P : (t + 1) * P],
                in_=pt4[:].rearrange("p (c q) -> p c q", c=DC),
            )
            # V projection for this token chunk, interleaved [V_h | 1] per head
            vps = lnp.tile([P, 512], F32, tag="vps")
            for dc in range(DC):
                nc.tensor.matmul(
                    vps,
                    xnT[:, dc, t * P : (t + 1) * P],
                    w_sb["wv"][:, dc, :],
                    start=(dc == 0), stop=(dc == DC - 1),
                )
            vslot = vaug[:, t, :].rearrange("p (h c) -> p h c", h=H)
            nc.vector.tensor_copy(
                out=vslot[:, :, 0:DK],
                in_=vps[:].rearrange("p (h c) -> p h c", h=H),
            )
            nc.vector.memset(vslot[:, :, DK : DK + 1], 1.0)
        # pair 0's Q/K projections before attention starts
        for g in qk_proj_groups(0, lnp):
            g()

    # ---------------- phase B/C/D: attention, pair-interleaved projections ---
    with tc.tile_pool(name="att", bufs=1, space="PSUM") as att:
        sc = [
            att.tile([P, NQ], F32, tag=f"sc{hs}", name=f"sc{hs}") for hs in (0, 1)
        ]
        for p in range(PAIRS):
            pvb = [
                att.tile([P, 512], F32, tag="pvb", bufs=3, name=f"pvb{p}_{j}")
                for j in range(3)
            ]
            pending = qk_proj_groups(p + 1, att) if p + 1 < PAIRS else []
            pts = [None] * NT
            for c in range(NT):
                # 4 score matmuls; hs-adjacent issue order so the two
                # 64-row tiles (rows 0-63 / 64-127) overlap in the array.
                for qg in range(NQ // 512):
                    for hs in range(2):
                        nc.tensor.matmul(
                            sc[hs][:, qg * 512 : (qg + 1) * 512],
                            kT[hs * DK : (hs + 1) * DK, p, c * P : (c + 1) * P],
                            qT[hs * DK : (hs + 1) * DK, p, qg * 512 : (qg + 1) * 512],
                            start=True, stop=True,
                        )
                pt = ptp.tile([P, 2 * NQ], BF16, tag="pt")
                pts[c] = pt
                for hs in range(2):
                    nc.scalar.activation(
                        out=pt[:, hs * NQ : (hs + 1) * NQ], in_=sc[hs],
                        func=mybir.ActivationFunctionType.Exp,
                        bias=mb_sb[:, c : c + 1], scale=1.0 / math.sqrt(DK),
                    )
                # PV for the previous chunk keeps PE busy under this exp
                if c > 0:
                    _pv_chunk(nc, pts[c - 1], vaug, pvb, p, c - 1)
                    pts[c - 1] = None
                # spread next pair's projections into the exp-shadow
                if pending and c >= 2 and c % 2 == 0:
                    pending.pop(0)()
            _pv_chunk(nc, pts[NT - 1], vaug, pvb, p, NT - 1)
            while pending:
                pending.pop(0)()

            # evacuate + normalize: l sits at slot col 64 (hs0) / 129 (hs1)
            rls = []
            for j, bank in enumerate(pvb):
                nslot = 3 if j < 2 else 2
                rl = rlp.tile([P, 3, 2], F32, tag="rl", name=f"rl{p}_{j}")
                lcols = bass.AP(
                    tensor=bank.tensor, offset=bank.offset + 64,
                    ap=[list(x) for x in bank.ap[:1]] + [[130, nslot], [65, 2]],
                )
                nc.vector.reciprocal(out=rl[:, :nslot, :], in_=lcols)
                rls.append(rl)
            for qt in range(NQT):
                bank = pvb[qt // 3]
                off = (qt % 3) * 130
                for hs in range(2):
                    nc.vector.tensor_scalar_mul(
                        out=attno[:, qt, (2 * p + hs) * DK : (2 * p + hs + 1) * DK],
                        in0=bank[:, off + hs * 65 : off + hs * 65 + DK],
                        scalar1=rls[qt // 3][:, qt % 3, hs : hs + 1],
                    )

    # ---------------- phase E: transpose + output projection -----------------
    with tc.tile_pool(name="projE", bufs=4, space="PSUM") as projE:
        for qt in range(NQT):
            pe = projE.tile([P, D], BF16, tag="eps")
            for c in range(DC):
                nc.tensor.transpose(
                    pe[:, c * P : (c + 1) * P], attno[:, qt, c * P : (c + 1) * P],
                    ident,
                )
            nc.vector.tensor_copy(
                out=outT[:, :, qt * P : (qt + 1) * P],
                in_=pe[:].rearrange("p (c q) -> p c q", c=DC),
            )
            po = projE.tile([P, D], F32, tag="ops")
            for dc in range(DC):
                nc.tensor.matmul(
                    po,
                    outT[:, dc, qt * P : (qt + 1) * P],
                    w_sb["wo"][:, dc, :],
                    start=(dc == 0), stop=(dc == DC - 1),
                )
            yt = yout.tile([P, D], F32, tag="yt")
            nc.vector.tensor_tensor(
                out=yt, in0=po, in1=bo_sb, op=mybir.AluOpType.add
            )
            nc.sync.dma_start(y_d[qt * P : (qt + 1) * P, :], yt)


def _pv_chunk(nc, pt, vaug, pvb, p, c):
    """P@[V|1] matmuls for chunk c of head-pair p: 8 query tiles x 2 heads,
    accumulated over chunks into the packed PSUM banks."""
    for qt in range(NQT):
        bank = pvb[qt // 3]
        off = (qt % 3) * 130
        for hs in range(2):
            h = 2 * p + hs
            # start=True clears has_written for the WHOLE bank, so only the
            # first packed region per bank may use it; the others rely on
            # overwrite-when-bit-clear for their first chunk.
            nc.tensor.matmul(
                bank[:, off + hs * 65 : off + (hs + 1) * 65],
                pt[:, hs * NQ + qt * P : hs * NQ + (qt + 1) * P],
                vaug[:, c, h * 65 : (h + 1) * 65],
                start=(c == 0 and qt % 3 == 0 and hs == 0),
                stop=(c == NT - 1),
                skip_group_check=True,
            )


_NC = None


def _get_nc():
    global _NC
    if _NC is None:
        from contextlib import ExitStack

        nc = bacc.Bacc(None, target_bir_lowering=False)
        with tile.TileContext(nc) as tc, ExitStack() as ctx:
            _emit(tc, ctx)
        nc.compile()
        _NC = nc
    return _NC


def kernel(
    inputs, input_lengths, pos_embed, ln_gamma, ln_beta,
    Wq, bq, Wk, bk, Wv, bv, Wo, bo,
):
    x = np.ascontiguousarray(np.asarray(inputs, np.float32))
    lengths = np.asarray(input_lengths, np.int32)
    g = np.asarray(ln_gamma, np.float32)
    be = np.asarray(ln_beta, np.float32)
    Wq = np.asarray(Wq, np.float32); bq = np.asarray(bq, np.float32)
    Wk = np.asarray(Wk, np.float32); bk = np.asarray(bk, np.float32)
    Wv = np.asarray(Wv, np.float32); bv = np.asarray(bv, np.float32)
    Wo = np.asarray(Wo, np.float32); bo = np.asarray(bo, np.float32)

    # Fold LayerNorm affine into the projections (exact: LN(x) = xh*g + be
    # with xh = (x-mu)*rstd, so LN(x)@W.T + b = xh@(g[:,None]*W.T) + (be@W.T + b)).
    wq_h = np.ascontiguousarray((g[:, None] * Wq.T).astype(np_bf16))
    wk_h = np.ascontiguousarray((g[:, None] * Wk.T).astype(np_bf16))
    wv_h = np.ascontiguousarray((g[:, None] * Wv.T).astype(np_bf16))
    wo_h = np.ascontiguousarray(Wo.T.astype(np_bf16))
    bq_h = np.ascontiguousarray((be @ Wq.T + bq).reshape(DC, P).T)
    bk_h = np.ascontiguousarray((be @ Wk.T + bk).reshape(DC, P).T)
    # V bias (incl. beta term) passes through softmax (rows sum to 1) and is
    # folded into the output-projection bias.
    bv_h = be @ Wv.T + bv
    bo_h = np.ascontiguousarray(bo + bv_h @ Wo.T)

    maskb = np.where(np.arange(S)[None, :] < lengths[:, None], 0.0, NEG).astype(
        np.float32
    )

    nc = _get_nc()
    in_maps = []
    core_assign = []
    for b in range(B):
        for gq in range(2):
            order = np.r_[gq * NQ : (gq + 1) * NQ, (1 - gq) * NQ : (2 - gq) * NQ]
            in_maps.append(
                {
                    "x": np.ascontiguousarray(x[b][order]),
                    "wq": wq_h, "wk": wk_h, "wv": wv_h, "wo": wo_h,
                    "bq": bq_h, "bk": bk_h, "bo": bo_h,
                    "maskb": np.ascontiguousarray(maskb[b][order].reshape(NT, P).T),
                }
            )
            core_assign.append((b, gq))

    global _LAST_IN_MAPS
    _LAST_IN_MAPS = in_maps
    res = run_bass_kernel_spmd(nc, in_maps, core_ids=list(range(8)))

    y = np.empty((B, S, D), np.float32)
    for i, (b, gq) in enumerate(core_assign):
        y[b, gq * NQ : (gq + 1) * NQ] = res.results[i]["y"]
    return y


# revision 4
# speedup vs baseline: 1.3424x; 1.1066x over previous
"""Trainium2 Bass kernel: MultiHeadSelfAttention (LayerNorm -> QKV -> masked
softmax attention -> output projection).

Problem shapes: B=4, S=2048, D=512, H=8, DK=64, fp32 I/O.

Sharding: 8 cores = 4 batches x 2 query-halves. Each core computes the full
K/V for its batch and attention outputs for its 1024 queries; no cross-core
communication. SPMD trick: the token order of each core's input is permuted on
the host so that the core's queries are always tokens 0..1023 (one static
program for all cores; attention is permutation-equivariant over keys as long
as the key-padding mask is permuted consistently).

Host prep: LayerNorm (memory-bound elementwise) + the [tok,d]->[d,tok]
transpose run in numpy, so the device receives xnT (bf16, d-major) and does
pure matmul/attention work. Weights are pre-transposed/cast to bf16 and the
V-bias is folded through softmax into the output bias.

Device schedule: the exp of all 16.8M score entries on ScalarE (~1ns/elem) is
the hard floor, so everything else is arranged to hide under it:
  - two persistent PSUM score tiles (one per head-half of the pair) let
    scores(c+1) overlap exp(c) with zero ACT stalls;
  - PV(c-1), V/QK projections for the next pair, and the output-side
    transposes of the finished pair all interleave into the PE queue to keep
    the PE dense (HAM stays at 2.4 GHz) and shrink the pre/post phases.

PSUM budget (8 banks): sc0 (2) + sc1 (2) + PV accumulators (3) + proj (1).
"""

import math

import numpy as np
from ml_dtypes import bfloat16 as np_bf16

import concourse.bass as bass
import concourse.tile as tile
from concourse import bacc, mybir
from concourse.bass_utils import run_bass_kernel_spmd
from concourse.masks import make_identity

B, S, D, H, DK = 4, 2048, 512, 8, 64
P = 128                 # partitions
NQ = 1024               # queries per core
NT = S // P             # 16 token tiles / key chunks
DC = D // P             # 4 d-chunks
NQT = NQ // P           # 8 query tiles
PAIRS = H // 2          # 4 head pairs
F32 = mybir.dt.float32
BF16 = mybir.dt.bfloat16
NEG = -1.0e30


def _emit(tc: tile.TileContext, ctx):
    nc = tc.nc

    xnT_d = nc.dram_tensor("xnT", [D, S], BF16, kind="ExternalInput")
    wq_d = nc.dram_tensor("wq", [D, D], BF16, kind="ExternalInput")
    wk_d = nc.dram_tensor("wk", [D, D], BF16, kind="ExternalInput")
    wv_d = nc.dram_tensor("wv", [D, D], BF16, kind="ExternalInput")
    wo_d = nc.dram_tensor("wo", [D, D], BF16, kind="ExternalInput")
    bq_d = nc.dram_tensor("bq", [P, DC], F32, kind="ExternalInput")
    bk_d = nc.dram_tensor("bk", [P, DC], F32, kind="ExternalInput")
    bo_d = nc.dram_tensor("bo", [D], F32, kind="ExternalInput")
    mb_d = nc.dram_tensor("maskb", [P, NT], F32, kind="ExternalInput")
    y_d = nc.dram_tensor("y", [NQ, D], F32, kind="ExternalOutput")

    consts = ctx.enter_context(tc.tile_pool(name="consts", bufs=1))
    big = ctx.enter_context(tc.tile_pool(name="big", bufs=1))
    ptp = ctx.enter_context(tc.tile_pool(name="ptp", bufs=4))
    rlp = ctx.enter_context(tc.tile_pool(name="rlp", bufs=6))
    yout = ctx.enter_context(tc.tile_pool(name="yout", bufs=3))

    ident = consts.tile([P, P], BF16, tag="ident")
    make_identity(nc, ident)
    bq_sb = consts.tile([P, DC], F32, tag="bq")
    nc.sync.dma_start(bq_sb, bq_d[:, :])
    bk_sb = consts.tile([P, DC], F32, tag="bk")
    nc.sync.dma_start(bk_sb, bk_d[:, :])
    mb_sb = consts.tile([P, NT], F32, tag="mb")
    nc.sync.dma_start(mb_sb, mb_d[:, :])
    bo_sb = consts.tile([P, D], F32, tag="bo")
    bo_ap = bo_d[:]
    nc.sync.dma_start(
        bo_sb, bass.AP(tensor=bo_ap.tensor, offset=bo_ap.offset, ap=[[0, P], [1, D]])
    )

    # persistent bf16 operands; DMA order = first-use order
    w_sb = {
        name: big.tile([P, DC, D], BF16, tag=f"{name}_sb", name=f"{name}_sb")
        for name in ("wq", "wk", "wv", "wo")
    }
    xnT = big.tile([P, DC, S], BF16, tag="xnT")
    for c in range(DC):
        nc.sync.dma_start(w_sb["wv"][:, c, :], wv_d[c * P : (c + 1) * P, :])
    for tg in range(S // 512):
        for c in range(DC):
            nc.sync.dma_start(
                xnT[:, c, tg * 512 : (tg + 1) * 512],
                xnT_d[c * P : (c + 1) * P, tg * 512 : (tg + 1) * 512],
            )
    for name in ("wq", "wk", "wo"):
        d = {"wq": wq_d, "wk": wk_d, "wo": wo_d}[name]
        for c in range(DC):
            nc.sync.dma_start(w_sb[name][:, c, :], d[c * P : (c + 1) * P, :])

    qT = big.tile([P, DC, NQ], BF16, tag="qT")
    kT = big.tile([P, DC, S], BF16, tag="kT")
    vaug = big.tile([P, NT, 8 * 65], BF16, tag="vaug")
    attno = big.tile([P, NQT, D], BF16, tag="attno")
    outT = big.tile([P, DC, NQ], BF16, tag="outT")

    def v_group(pool, t):
        def emit():
            vps = pool.tile([P, 512], F32, tag="pp", name=f"vps{t}")
            for dc in range(DC):
                nc.tensor.matmul(
                    vps,
                    xnT[:, dc, t * P : (t + 1) * P],
                    w_sb["wv"][:, dc, :],
                    start=(dc == 0), stop=(dc == DC - 1),
                )
            vslot = vaug[:, t, :].rearrange("p (h c) -> p h c", h=H)
            nc.vector.tensor_copy(
                out=vslot[:, :, 0:DK],
                in_=vps[:].rearrange("p (h c) -> p h c", h=H),
            )
            nc.vector.memset(vslot[:, :, DK : DK + 1], 1.0)
        return emit

    def qk_groups(pool, p):
        """Emit-closures for pair p's Q and K projections (d-chunk p)."""
        groups = []

        def q_group(qg):
            def emit():
                ps = pool.tile([P, 512], F32, tag="pp", name=f"qps{p}_{qg}")
                for dc in range(DC):
                    nc.tensor.matmul(
                        ps,
                        w_sb["wq"][:, dc, p * P : (p + 1) * P],
                        xnT[:, dc, qg * 512 : (qg + 1) * 512],
                        start=(dc == 0), stop=(dc == DC - 1),
                    )
                nc.vector.tensor_scalar_add(
                    out=qT[:, p, qg * 512 : (qg + 1) * 512], in0=ps,
                    scalar1=bq_sb[:, p : p + 1],
                )
            return emit

        def k_group(kg):
            def emit():
                ps = pool.tile([P, 512], F32, tag="pp", name=f"kps{p}_{kg}")
                for dc in range(DC):
                    nc.tensor.matmul(
                        ps,
                        w_sb["wk"][:, dc, p * P : (p + 1) * P],
                        xnT[:, dc, kg * 512 : (kg + 1) * 512],
                        start=(dc == 0), stop=(dc == DC - 1),
                    )
                nc.vector.tensor_scalar_add(
                    out=kT[:, p, kg * 512 : (kg + 1) * 512], in0=ps,
                    scalar1=bk_sb[:, p : p + 1],
                )
            return emit

        for qg in range(NQ // 512):
            groups.append(q_group(qg))
        for kg in range(S // 512):
            groups.append(k_group(kg))
        return groups

    # ---------------- attention, everything else in its shadow ----------------
    with tc.tile_pool(name="att", bufs=1, space="PSUM") as att:
        # prologue: V for the first chunks + pair-0 Q/K
        for t in range(2):
            v_group(att, t)()
        for g in qk_groups(att, 0):
            g()

        sc = [
            att.tile([P, NQ], F32, tag=f"sc{hs}", name=f"sc{hs}") for hs in (0, 1)
        ]
        for p in range(PAIRS):
            pvb = [
                att.tile([P, 512], F32, tag="pvb", bufs=3, name=f"pvb{p}_{j}")
                for j in range(3)
            ]
            pending = []
            if p == 0:
                pending += [v_group(att, t) for t in range(2, NT)]
            if p + 1 < PAIRS:
                pending += qk_groups(att, p + 1)
            pts = [None] * NT
            for c in range(NT):
                # 4 score matmuls; hs-adjacent issue order so the two
                # 64-row tiles (rows 0-63 / 64-127) overlap in the array.
                for qg in range(NQ // 512):
                    for hs in range(2):
                        nc.tensor.matmul(
                            sc[hs][:, qg * 512 : (qg + 1) * 512],
                            kT[hs * DK : (hs + 1) * DK, p, c * P : (c + 1) * P],
                            qT[hs * DK : (hs + 1) * DK, p, qg * 512 : (qg + 1) * 512],
                            start=True, stop=True,
                        )
                pt = ptp.tile([P, 2 * NQ], BF16, tag="pt")
                pts[c] = pt
                for hs in range(2):
                    nc.scalar.activation(
                        out=pt[:, hs * NQ : (hs + 1) * NQ], in_=sc[hs],
                        func=mybir.ActivationFunctionType.Exp,
                        bias=mb_sb[:, c : c + 1], scale=1.0 / math.sqrt(DK),
                    )
                # PV for the previous chunk keeps PE busy under this exp
                if c > 0:
                    _pv_chunk(nc, pts[c - 1], vaug, pvb, p, c - 1)
                    pts[c - 1] = None
                # spread deferred projection work into the exp-shadow
                if pending:
                    pending.pop(0)()
            _pv_chunk(nc, pts[NT - 1], vaug, pvb, p, NT - 1)
            while pending:
                pending.pop(0)()

            # evacuate + normalize: l sits at slot col 64 (hs0) / 129 (hs1);
            # bank-ordered so pair p+1's PV can reclaim banks incrementally.
            rls = []
            for j, bank in enumerate(pvb):
                nslot = 3 if j < 2 else 2
                rl = rlp.tile([P, 3, 2], F32, tag="rl", name=f"rl{p}_{j}")
                lcols = bass.AP(
                    tensor=bank.tensor, offset=bank.offset + 64,
                    ap=[list(x) for x in bank.ap[:1]] + [[130, nslot], [65, 2]],
                )
                nc.vector.reciprocal(out=rl[:, :nslot, :], in_=lcols)
                rls.append(rl)
            for qt in range(NQT):
                bank = pvb[qt // 3]
                off = (qt % 3) * 130
                for hs in range(2):
                    nc.vector.tensor_scalar_mul(
                        out=attno[:, qt, (2 * p + hs) * DK : (2 * p + hs + 1) * DK],
                        in0=bank[:, off + hs * 65 : off + hs * 65 + DK],
                        scalar1=rls[qt // 3][:, qt % 3, hs : hs + 1],
                    )
            # transpose this pair's attention-output columns (d-chunk p)
            for qt in range(NQT):
                tre = att.tile([P, P], BF16, tag="pp", name=f"tre{p}_{qt}")
                nc.tensor.transpose(
                    tre, attno[:, qt, p * P : (p + 1) * P], ident
                )
                nc.vector.tensor_copy(
                    out=outT[:, p, qt * P : (qt + 1) * P], in_=tre
                )

    # ---------------- output projection -----------------
    with tc.tile_pool(name="projE", bufs=4, space="PSUM") as projE:
        for qt in range(NQT):
            po = projE.tile([P, D], F32, tag="ops")
            for dc in range(DC):
                nc.tensor.matmul(
                    po,
                    outT[:, dc, qt * P : (qt + 1) * P],
                    w_sb["wo"][:, dc, :],
                    start=(dc == 0), stop=(dc == DC - 1),
                )
            yt = yout.tile([P, D], F32, tag="yt")
            nc.vector.tensor_tensor(
                out=yt, in0=po, in1=bo_sb, op=mybir.AluOpType.add
            )
            nc.sync.dma_start(y_d[qt * P : (qt + 1) * P, :], yt)


def _pv_chunk(nc, pt, vaug, pvb, p, c):
    """P@[V|1] matmuls for chunk c of head-pair p: 8 query tiles x 2 heads,
    accumulated over chunks into the packed PSUM banks."""
    for qt in range(NQT):
        bank = pvb[qt // 3]
        off = (qt % 3) * 130
        for hs in range(2):
            h = 2 * p + hs
            # start=True clears has_written for the WHOLE bank, so only the
            # first packed region per bank may use it; the others rely on
            # overwrite-when-bit-clear for their first chunk.
            nc.tensor.matmul(
                bank[:, off + hs * 65 : off + (hs + 1) * 65],
                pt[:, hs * NQ + qt * P : hs * NQ + (qt + 1) * P],
                vaug[:, c, h * 65 : (h + 1) * 65],
                start=(c == 0 and qt % 3 == 0 and hs == 0),
                stop=(c == NT - 1),
                skip_group_check=True,
            )


_NC = None


def _get_nc():
    global _NC
    if _NC is None:
        from contextlib import ExitStack

        nc = bacc.Bacc(None, target_bir_lowering=False)
        with tile.TileContext(nc) as tc, ExitStack() as ctx:
            _emit(tc, ctx)
        nc.compile()
        _NC = nc
    return _NC


def kernel(
    inputs, input_lengths, pos_embed, ln_gamma, ln_beta,
    Wq, bq, Wk, bk, Wv, bv, Wo, bo,
):
    x = np.ascontiguousarray(np.asarray(inputs, np.float32))
    lengths = np.asarray(input_lengths, np.int32)
    g = np.asarray(ln_gamma, np.float32)
    be = np.asarray(ln_beta, np.float32)
    Wq = np.asarray(Wq, np.float32); bq = np.asarray(bq, np.float32)
    Wk = np.asarray(Wk, np.float32); bk = np.asarray(bk, np.float32)
    Wv = np.asarray(Wv, np.float32); bv = np.asarray(bv, np.float32)
    Wo = np.asarray(Wo, np.float32); bo = np.asarray(bo, np.float32)

    # LayerNorm on host (eps=1e-5), fp32, then bf16 d-major per core.
    mu = x.mean(-1, keepdims=True)
    xc = x - mu
    var = np.mean(xc * xc, axis=-1, keepdims=True)
    xn = (xc / np.sqrt(var + 1e-5)) * g + be

    wq_h = np.ascontiguousarray(Wq.T.astype(np_bf16))
    wk_h = np.ascontiguousarray(Wk.T.astype(np_bf16))
    wv_h = np.ascontiguousarray(Wv.T.astype(np_bf16))
    wo_h = np.ascontiguousarray(Wo.T.astype(np_bf16))
    bq_h = np.ascontiguousarray(bq.reshape(DC, P).T)
    bk_h = np.ascontiguousarray(bk.reshape(DC, P).T)
    # V bias passes through softmax (rows sum to 1) -> fold into output bias.
    bo_h = np.ascontiguousarray(bo + bv @ Wo.T)

    maskb = np.where(np.arange(S)[None, :] < lengths[:, None], 0.0, NEG).astype(
        np.float32
    )

    nc = _get_nc()
    in_maps = []
    core_assign = []
    for b in range(B):
        for gq in range(2):
            order = np.r_[gq * NQ : (gq + 1) * NQ, (1 - gq) * NQ : (2 - gq) * NQ]
            in_maps.append(
                {
                    "xnT": np.ascontiguousarray(xn[b][order].T.astype(np_bf16)),
                    "wq": wq_h, "wk": wk_h, "wv": wv_h, "wo": wo_h,
                    "bq": bq_h, "bk": bk_h, "bo": bo_h,
                    "maskb": np.ascontiguousarray(maskb[b][order].reshape(NT, P).T),
                }
            )
            core_assign.append((b, gq))

    global _LAST_IN_MAPS
    _LAST_IN_MAPS = in_maps
    res = run_bass_kernel_spmd(nc, in_maps, core_ids=list(range(8)))

    y = np.empty((B, S, D), np.float32)
    for i, (b, gq) in enumerate(core_assign):
        y[b, gq * NQ : (gq + 1) * NQ] = res.results[i]["y"]
    return y
